# revision 20
# baseline (speedup 1.0000x reference)
"""Trainium2 Bass kernel for CrossAttentionPlus.

Math (reference):
    q,k,v = proj(query,key,value); scores = q@k^T * D**-0.5
    scores = where(causal, -1e9, scores) + attention_mask
    local = softmax(scores); attn = 0.5*local + 0.5*supplied
    attn = attn / (attn.sum(-1) + 1e-9); attn = where(causal, 0, attn)
    out = (attn @ v) @ Wo + bo

Sharding: 8 cores; core c handles batch b=c//2 and heads [8*(c%2), 8*(c%2)+8).
Each core returns a partial output [Q, DIM]; host sums the two head-half
partials per batch and adds bo.

Device algorithm (per core), f16 matmuls (1 col/cycle @2.4GHz) except the
supplied branch which runs fp8e4 DoubleRow (2 k-blocks per pass):
    - Projections consume host-transposed activations (x^T: [DIM, tok]) so
      Q^T [c,q], K^T [c,q] and V [k,c] come out of the PE in natural layout.
    - Attention runs in transposed layout S^T [k, q]; exp(S^T) is the moving
      operand for attn@V with V stationary; a ones-column appended to V
      accumulates E_q = sum_k exp for free.
    - Causal diag masking is an additive PE matmul (ident @ (-65504*tril))
      accumulated into the score PSUM before exp -> no DVE dependency.
    - supplied^T arrives as fp8e4, pre-scaled by 256*c2 and causally zeroed;
      sup@V uses DoubleRow (paired k-blocks) with an fp8 copy of V; the
      x256 is undone by passing Wo/256 (AT tiles carry 256x values).
    - Normalization: c1 = 256*c2/E via DVE row ops + gpsimd broadcast.
    - Phase interleaving: V-proj kb4-7 rides inside attention qc0; output
      projection for q<512 rides inside attention qc1; scores are issued two
      k-blocks ahead of their consumers to hide the scalar-engine exp.
"""

import numpy as np
from contextlib import ExitStack

B, Q, KLEN, DIM, H, D = 4, 1024, 1024, 1024, 16, 64
SCALE = float(D) ** -0.5
MIX = 0.5
NEG = -1.0e9
N_CORES = 8
NH = 8            # heads per core
P = 128
NKB = KLEN // P   # 8 k-blocks
QCH = 512         # q chunk (one PSUM bank of fp32)
SUPS = 256.0      # supplied-branch pre-scale (undone via Wo/SUPS)

_BUILD_CACHE = {}


def _build(causal: bool):
    """Build + compile the Bass program. causal=True: standard causal mask;
    causal=False: no masking at all."""
    import concourse.tile as tile
    import concourse.mybir as mybir
    from concourse import bacc

    F32 = mybir.dt.float32
    F16 = mybir.dt.float16
    F8 = mybir.dt.float8e4
    AF = mybir.ActivationFunctionType
    OP = mybir.AluOpType
    DR = mybir.MatmulPerfMode.DoubleRow

    nc = bacc.Bacc("TRN2", target_bir_lowering=False, debug=False,
                   num_devices=N_CORES)

    qT = nc.dram_tensor("qT", [DIM, Q], F16, kind="ExternalInput").ap()
    kT = nc.dram_tensor("kT", [DIM, KLEN], F16, kind="ExternalInput").ap()
    vT = nc.dram_tensor("vT", [DIM, KLEN], F16, kind="ExternalInput").ap()
    wq = nc.dram_tensor("wq", [DIM, NH * D], F16, kind="ExternalInput").ap()
    wk = nc.dram_tensor("wk", [DIM, NH * D], F16, kind="ExternalInput").ap()
    wv = nc.dram_tensor("wv", [DIM, NH * D], F16, kind="ExternalInput").ap()
    wo = nc.dram_tensor("wo", [NH * D, DIM], F16, kind="ExternalInput").ap()
    sup = nc.dram_tensor("sup", [NH, KLEN, Q], F8, kind="ExternalInput").ap()
    c2 = nc.dram_tensor("c2", [NH, Q], F32, kind="ExternalInput").ap()
    tri = nc.dram_tensor("tri", [P, P], F16, kind="ExternalInput").ap()
    idn = nc.dram_tensor("idn", [P, P], F16, kind="ExternalInput").ap()
    out = nc.dram_tensor("out_p", [Q, DIM], F16, kind="ExternalOutput").ap()

    def wlo_of(kb, qc):
        # start column of k-block kb's unmasked window, relative to chunk qc.
        if not causal:
            return 0
        return max(P * kb - qc * QCH, 0)

    with tile.TileContext(nc) as tc:
        with ExitStack() as ctx:
            # --- pools ---
            xT_pool = ctx.enter_context(tc.tile_pool(name="xT", bufs=3))
            w_pool = ctx.enter_context(tc.tile_pool(name="w", bufs=2))
            st_pool = ctx.enter_context(tc.tile_pool(name="store", bufs=1))
            sup_pool = ctx.enter_context(tc.tile_pool(name="sup", bufs=6))
            exp_pool = ctx.enter_context(tc.tile_pool(name="exp", bufs=5))
            row_pool = ctx.enter_context(tc.tile_pool(name="rows", bufs=2))
            rep_pool = ctx.enter_context(tc.tile_pool(name="rep", bufs=2))
            tmp_pool = ctx.enter_context(tc.tile_pool(name="tmp", bufs=2))
            const_pool = ctx.enter_context(tc.tile_pool(name="const", bufs=1))
            outb_pool = ctx.enter_context(tc.tile_pool(name="outb", bufs=2))

            # --- constants ---
            warm = const_pool.tile([P, QCH], F16, tag="warm")
            nc.vector.memset(warm[:], 0.0)
            if causal:
                tri_sb = const_pool.tile([P, P], F16, tag="tri")
                nc.sync.dma_start(tri_sb[:], tri)
                idn_sb = const_pool.tile([P, P], F16, tag="idn")
                nc.sync.dma_start(idn_sb[:], idn)

            # --- persistent stores ---
            # QT_st/KT_st tile j holds projected heads 2j,2j+1: [c=128, q=1024]
            QT_st = [st_pool.tile([P, Q], F16, tag=f"qt{j}", name=f"qt{j}")
                     for j in range(4)]
            KT_st = [st_pool.tile([P, Q], F16, tag=f"kt{j}", name=f"kt{j}")
                     for j in range(4)]
            # V_st[kb]: [k=128, NH*(D+1)] f16 (per head: D cols of V, ones col)
            V_st = [st_pool.tile([P, NH * (D + 1)], F16, tag=f"vst{kb}",
                                 name=f"vst{kb}") for kb in range(NKB)]
            # V8_st[pb]: [k=128, 2, NH*D] fp8 pair tiles for DoubleRow
            V8_st = [st_pool.tile([P, 2, NH * D], F8, tag=f"v8st{pb}",
                                  name=f"v8st{pb}") for pb in range(NKB // 2)]
            # attnT tile j: [hd=128 (heads 2j,2j+1), q=1024], carries 256x
            AT_st = [st_pool.tile([P, Q], F16, tag=f"at{j}", name=f"at{j}")
                     for j in range(4)]

            # ========== Phase 1a: Q/K projections ==========
            # ib-outer: 8 concurrent PSUM chains (all banks) so each DMA
            # chunk arrival immediately feeds 8 matmuls -> PE tracks the
            # cold-start DMA stream with minimal idle.
            with ExitStack() as pctx:
                proj_psum = pctx.enter_context(
                    tc.tile_pool(name="projpsum", bufs=8, space="PSUM"))
                # PE warmup (ramp the p-state while the first chunks land)
                wps = proj_psum.tile([P, QCH], F32, tag="c", name="wps")
                for _ in range(4):
                    nc.tensor.matmul(wps[:], warm[:, 0:P], warm[:],
                                     start=True, stop=True)
                for name, w_ap, x_ap, dst in (
                    ("q", wq, qT, QT_st), ("k", wk, kT, KT_st)):
                    w_sb = w_pool.tile([P, NKB, NH * D], F16, tag="w")
                    w_r = w_ap.rearrange("(n p) c -> p n c", p=P)
                    x_r = x_ap.rearrange("(n p) q -> p n q", p=P)
                    xh = [xT_pool.tile([P, 4, Q], F16, tag="xT",
                                       name=f"x{name}{half}")
                          for half in range(2)]
                    for ib in range(NKB):
                        nc.sync.dma_start(w_sb[:, ib:ib + 1, :],
                                          w_r[:, ib:ib + 1, :])
                        nc.sync.dma_start(
                            xh[ib // 4][:, ib % 4:ib % 4 + 1, :],
                            x_r[:, ib:ib + 1, :])
                    chains = [proj_psum.tile([P, QCH], F32, tag="c",
                                             name=f"{name}c{i}")
                              for i in range(8)]
                    for ib in range(NKB):
                        for ct in range(4):
                            for qc in range(2):
                                nc.tensor.matmul(
                                    chains[ct * 2 + qc][:],
                                    w_sb[:, ib, ct * P:(ct + 1) * P],
                                    xh[ib // 4][:, ib % 4,
                                                qc * QCH:(qc + 1) * QCH],
                                    start=(ib == 0), stop=(ib == NKB - 1))
                    for ct in range(4):
                        for qc in range(2):
                            nc.vector.tensor_copy(
                                out=dst[ct][:, qc * QCH:(qc + 1) * QCH],
                                in_=chains[ct * 2 + qc][:])

            s_psum = ctx.enter_context(
                tc.tile_pool(name="spsum", bufs=3, space="PSUM"))
            a_psum = ctx.enter_context(
                tc.tile_pool(name="apsum", bufs=2, space="PSUM"))
            b_psum = ctx.enter_context(
                tc.tile_pool(name="bpsum", bufs=2, space="PSUM"))

            # ========== Phase 1b: V projection (split around attention) ==
            wv_sb = w_pool.tile([P, NKB, NH * D], F16, tag="w")
            wv_r = wv.rearrange("(n p) c -> p n c", p=P)
            vT_r = vT.rearrange("(n p) q -> p n q", p=P)
            vh = [xT_pool.tile([P, 4, KLEN], F16, tag="xT", name=f"xv{half}")
                  for half in range(2)]
            for ib in range(NKB):
                nc.sync.dma_start(wv_sb[:, ib:ib + 1, :], wv_r[:, ib:ib + 1, :])
                nc.sync.dma_start(
                    vh[ib // 4][:, ib % 4:ib % 4 + 1, :],
                    vT_r[:, ib:ib + 1, :])

            def v_copies(kb, ps):
                # scatter per-head 64-col groups into the 65-stride layout
                nc.scalar.copy(
                    V_st[kb][:].rearrange("p (h x) -> p h x", x=D + 1)[:, :, 0:D],
                    ps[:].rearrange("p (h x) -> p h x", x=D))
                nc.vector.memset(
                    V_st[kb][:].rearrange("p (h x) -> p h x", x=D + 1)[:, :, D:D + 1],
                    1.0)
                nc.scalar.copy(V8_st[kb // 2][:, kb % 2, :], ps[:])

            def v_proj(kb):
                # V: out[k, c] += v^T[i, k]-as-lhsT @ Wv[i, c]
                ps = s_psum.tile([P, NH * D], F32, tag="s")
                for ib in range(NKB):
                    nc.tensor.matmul(
                        ps[:],
                        vh[ib // 4][:, ib % 4, kb * P:(kb + 1) * P],
                        wv_sb[:, ib, :],
                        start=(ib == 0), stop=(ib == NKB - 1))
                v_copies(kb, ps)

            def v_proj_pre4():
                # ib-outer over kb0-3: 4 concurrent chains (a/b banks are
                # idle before attention) track the vT DMA chunk arrivals
                vch = [s_psum.tile([P, NH * D], F32, tag="s", name="vch0"),
                       s_psum.tile([P, NH * D], F32, tag="s", name="vch1"),
                       a_psum.tile([P, NH * D], F32, tag="o2a", name="vch2"),
                       b_psum.tile([P, NH * D], F32, tag="o2b", name="vch3")]
                for ib in range(NKB):
                    for kb in range(4):
                        nc.tensor.matmul(
                            vch[kb][:],
                            vh[ib // 4][:, ib % 4, kb * P:(kb + 1) * P],
                            wv_sb[:, ib, :],
                            start=(ib == 0), stop=(ib == NKB - 1))
                for kb in range(4):
                    v_copies(kb, vch[kb])

            # wo load early so phase 3 never waits
            wo_sb = w_pool.tile([P, 4, DIM], F16, tag="wow", name="wo_sb")
            nc.sync.dma_start(
                wo_sb[:], wo.rearrange("(n p) o -> p n o", p=P))

            n_vpre = 4 if causal else NKB
            if causal:
                v_proj_pre4()
            else:
                for kb in range(n_vpre):
                    v_proj(kb)

            # ========== Phase 2: attention ==========
            sup_r = sup.rearrange("h (n p) q -> h p n q", p=P)

            def attn_head(h, qc):
                j, po = h // 2, (h % 2) * D
                kmax = (4 * qc + 4) if causal else NKB
                npairs = kmax // 2
                cols = slice(qc * QCH, (qc + 1) * QCH)
                # supplied^T fp8 load: k-block pairs with causal windows
                sup_t = sup_pool.tile([P, kmax, QCH], F8, tag="sup")
                if causal and qc == 0:
                    for kb2 in range(0, 4, 2):
                        w2 = wlo_of(kb2, 0)
                        nc.sync.dma_start(
                            sup_t[:, kb2:kb2 + 2, w2:],
                            sup_r[h, :, kb2:kb2 + 2, w2:QCH])
                else:
                    nc.sync.dma_start(
                        sup_t[:, 0:kmax, :], sup_r[h, :, 0:kmax, cols])
                c2row = row_pool.tile([1, QCH], F32, tag="c2row")
                nc.sync.dma_start(c2row[:], c2[h:h + 1, cols])

                o2a = a_psum.tile([D + 1, QCH], F32, tag="o2a")
                o2b = b_psum.tile([D, QCH], F32, tag="o2b")

                s_ps = [None] * kmax
                e_t = [None] * kmax

                def score(kb):
                    wlo = wlo_of(kb, qc)
                    ps = s_psum.tile([P, QCH], F32, tag="s", name=f"sps{kb}")
                    s_ps[kb] = ps
                    diag = causal and (4 * qc <= kb < 4 * qc + 4)
                    nc.tensor.matmul(
                        ps[:, wlo:],
                        KT_st[j][po:po + D, kb * P:(kb + 1) * P],
                        QT_st[j][po:po + D, qc * QCH + wlo:(qc + 1) * QCH],
                        start=True, stop=not diag)
                    if diag:
                        dstart = kb * P - qc * QCH
                        nc.tensor.matmul(
                            ps[:, dstart:dstart + P], idn_sb[:], tri_sb[:],
                            start=False, stop=True, skip_group_check=True)
                    et = exp_pool.tile([P, QCH], F16, tag="e", name=f"et{kb}")
                    e_t[kb] = et
                    nc.scalar.activation(
                        et[:, wlo:], ps[:, wlo:], AF.Exp, bias=0.0, scale=SCALE)

                def o2a_mm(kb):
                    wlo = wlo_of(kb, qc)
                    nc.tensor.matmul(
                        o2a[:, wlo:],
                        V_st[kb][:, h * (D + 1):(h + 1) * (D + 1)],
                        e_t[kb][:, wlo:],
                        start=(kb == 0), stop=(kb == kmax - 1))
                    s_ps[kb] = None
                    e_t[kb] = None

                def o2b_mm(pb):
                    pw = wlo_of(2 * pb, qc)
                    nc.tensor.matmul(
                        o2b[:, pw:],
                        V8_st[pb][:, :, h * D:(h + 1) * D],
                        sup_t[:, 2 * pb:2 * pb + 2, pw:],
                        start=(pb == 0), stop=(pb == npairs - 1),
                        perf_mode=DR)

                # software pipeline: scores two blocks ahead; sup pairs fill
                # the exp latency between score(kb) and o2a(kb).
                score(0)
                score(1)
                for kb in range(kmax):
                    if kb % 2 == 0 and kb // 2 < npairs:
                        o2b_mm(kb // 2)
                    o2a_mm(kb)
                    if kb + 2 < kmax:
                        score(kb + 2)

                # c1 = 256*c2 / E ; attn^T = c1 (x) o2a[0:D] + o2b
                # (reciprocal_approx_fast reads garbage from PSUM; stage via
                # SBUF first)
                ecopy = row_pool.tile([1, QCH], F32, tag="ecopy")
                nc.vector.tensor_copy(out=ecopy[:], in_=o2a[D:D + 1, :])
                erec = row_pool.tile([1, QCH], F32, tag="erec")
                nc.vector.reciprocal_approx_fast(erec[:], ecopy[:])
                c1r = row_pool.tile([1, QCH], F32, tag="c1r")
                nc.vector.tensor_tensor(
                    out=c1r[:], in0=erec[:], in1=c2row[:], op=OP.mult)
                rep = rep_pool.tile([D, QCH], F32, tag="rep")
                nc.gpsimd.partition_broadcast(rep[:], c1r[:])
                t1 = tmp_pool.tile([D, QCH], F32, tag="t1")
                nc.vector.tensor_tensor(
                    out=t1[:], in0=o2a[0:D, :], in1=rep[:], op=OP.mult)
                nc.vector.tensor_tensor(
                    out=AT_st[j][po:po + D, cols], in0=t1[:],
                    in1=o2b[:], op=OP.add)

            # ========== Phase 3: output projection (per q-half) ==========
            def phase3_chunk(m, oc):
                ps = s_psum.tile([P, QCH], F32, tag="s")
                for jj in range(4):
                    nc.tensor.matmul(
                        ps[:],
                        AT_st[jj][:, m * P:(m + 1) * P],
                        wo_sb[:, jj, oc * QCH:(oc + 1) * QCH],
                        start=(jj == 0), stop=(jj == 3))
                ob = outb_pool.tile([P, QCH], F16, tag="ob")
                # alternate copy engine so the final drain is not serialized
                # on the scalar engine
                if (m + oc) % 2 == 0:
                    nc.scalar.copy(ob[:], ps[:])
                else:
                    nc.vector.tensor_copy(out=ob[:], in_=ps[:])
                nc.sync.dma_start(
                    out[m * P:(m + 1) * P, oc * QCH:(oc + 1) * QCH], ob[:])

            def phase3(qc):
                for m in range(4 * qc, 4 * qc + 4):
                    for oc in range(2):
                        phase3_chunk(m, oc)

            # attention qc0 with the tail of the V projection interleaved
            for h in range(NH):
                attn_head(h, 0)
                if causal and h < NKB - n_vpre:
                    v_proj(n_vpre + h)
            # attention qc1; output projection for q<512 spread across head
            # boundaries to smooth engine power (dense bursts trip the
            # throttle and drop the PE clock)
            # 6 chunks spread over h1..h6; 2 reserved to cover the last
            # head's c1-chain latency before phase3(1) can start
            p3q0 = [(m, oc) for m in range(4) for oc in range(2)]
            for h in range(NH):
                attn_head(h, 1)
                if 1 <= h <= 6:
                    phase3_chunk(*p3q0.pop(0))
            while p3q0:
                phase3_chunk(*p3q0.pop(0))
            phase3(1)

    nc.compile()
    return nc


def _prep_inputs(query, key, value, supplied_attn, Wq, Wk, Wv, Wo, causal):
    """Host-side marshaling: per-core transposed slices + normalization rows."""
    import ml_dtypes
    f32 = np.float32
    f16 = np.float16
    f8 = ml_dtypes.float8_e4m3
    # c2 = MIX / (MIX*sum(local) + (1-MIX)*sum(supplied) + 1e-9); sum(local)=1
    s_row = supplied_attn.sum(axis=-1, dtype=np.float32)          # [B,H,Q]
    denom = (MIX + (1.0 - MIX) * s_row + 1e-9).astype(f32)
    c2f = (np.float32(1.0 - MIX) / denom).astype(f32)             # [B,H,Q]
    c2_exp = (np.float32(MIX * SUPS) / denom).astype(f32)         # exp branch

    trif = np.tril(np.full((P, P), -65504.0, dtype=f16), -1)      # k>q rows
    idnf = np.eye(P, dtype=f16)

    in_maps = []
    for core in range(N_CORES):
        b, hh = core // 2, core % 2
        h0 = hh * NH
        qTa = np.ascontiguousarray(query[b].T.astype(f16))
        kTa = np.ascontiguousarray(key[b].T.astype(f16))
        vTa = np.ascontiguousarray(value[b].T.astype(f16))
        wqa = np.ascontiguousarray(Wq[:, h0 * D:(h0 + NH) * D].astype(f16))
        wka = np.ascontiguousarray(Wk[:, h0 * D:(h0 + NH) * D].astype(f16))
        wva = np.ascontiguousarray(Wv[:, h0 * D:(h0 + NH) * D].astype(f16))
        woa = np.ascontiguousarray(
            (Wo[h0 * D:(h0 + NH) * D, :] * (1.0 / SUPS)).astype(f16))
        s = supplied_attn[b, h0:h0 + NH]                          # [NH, Q, K]
        s = s * (c2f[b, h0:h0 + NH, :, None] * np.float32(SUPS))  # pre-scale
        if causal:
            s = np.tril(s)                                        # zero k>q
        supa = np.ascontiguousarray(s.transpose(0, 2, 1).astype(f8))  # [NH,K,Q]
        in_maps.append({
            "qT": qTa, "kT": kTa, "vT": vTa,
            "wq": wqa, "wk": wka, "wv": wva, "wo": woa,
            "sup": supa,
            "c2": np.ascontiguousarray(c2_exp[b, h0:h0 + NH], dtype=f32),
            "tri": trif, "idn": idnf,
        })
    return in_maps


def _fallback_numpy(query, key, value, attention_mask, supplied_attn,
                    Wq, Wk, Wv, Wo, bo, causal_mask):
    q = (query @ Wq).reshape(B, Q, H, D).transpose(0, 2, 1, 3)
    k = (key @ Wk).reshape(B, KLEN, H, D).transpose(0, 2, 1, 3)
    v = (value @ Wv).reshape(B, KLEN, H, D).transpose(0, 2, 1, 3)
    scores = np.einsum("bhqd,bhkd->bhqk", q, k).astype(np.float32) * np.float32(SCALE)
    cm = np.broadcast_to(causal_mask, scores.shape)
    scores = np.where(cm, np.float32(NEG), scores)
    scores = scores + attention_mask
    m = scores.max(axis=-1, keepdims=True)
    e = np.exp(scores - m)
    local = e / e.sum(axis=-1, keepdims=True)
    attn = np.float32(MIX) * local + np.float32(1.0 - MIX) * supplied_attn
    attn = attn / (attn.sum(axis=-1, keepdims=True) + np.float32(1e-9))
    attn = np.where(cm, np.float32(0.0), attn)
    o = np.einsum("bhqk,bhkd->bhqd", attn, v)
    o = o.transpose(0, 2, 1, 3).reshape(B, Q, H * D)
    return (o @ Wo + bo).astype(np.float32)


def kernel(query, key, value, attention_mask, supplied_attn,
           Wq, Wk, Wv, Wo, bo, causal_mask, _collect=None):
    query = np.asarray(query); key = np.asarray(key); value = np.asarray(value)
    attention_mask = np.asarray(attention_mask)
    supplied_attn = np.asarray(supplied_attn)
    Wq = np.asarray(Wq); Wk = np.asarray(Wk); Wv = np.asarray(Wv)
    Wo = np.asarray(Wo); bo = np.asarray(bo)
    causal_mask = np.asarray(causal_mask)

    cm2 = causal_mask.reshape(causal_mask.shape[-2], causal_mask.shape[-1])
    is_std_causal = bool(
        np.array_equal(cm2, np.triu(np.ones((Q, KLEN), dtype=bool), 1)))
    is_no_mask = not causal_mask.any()
    if attention_mask.any() or not (is_std_causal or is_no_mask):
        return _fallback_numpy(query, key, value, attention_mask,
                               supplied_attn, Wq, Wk, Wv, Wo, bo, causal_mask)

    import concourse.bass_utils as bass_utils
    causal = is_std_causal
    key_ = ("causal" if causal else "nomask")
    if key_ not in _BUILD_CACHE:
        _BUILD_CACHE[key_] = _build(causal)
    nc = _BUILD_CACHE[key_]

    in_maps = _prep_inputs(query, key, value, supplied_attn, Wq, Wk, Wv, Wo,
                           causal)
    run_kwargs = dict(_collect) if _collect else {}
    res = bass_utils.run_bass_kernel_spmd(
        nc, in_maps, core_ids=list(range(N_CORES)), **run_kwargs)
    if _collect is not None:
        _collect["results"] = res

    out = np.empty((B, Q, DIM), dtype=np.float32)
    for b in range(B):
        out[b] = (res.results[2 * b]["out_p"].astype(np.float32)
                  + res.results[2 * b + 1]["out_p"].astype(np.float32)
                  + bo.astype(np.float32))
    return out


# revision 22
# speedup vs baseline: 1.0484x; 1.0484x over previous
"""Trainium2 Bass kernel for CrossAttentionPlus.

Math (reference):
    q,k,v = proj(query,key,value); scores = q@k^T * D**-0.5
    scores = where(causal, -1e9, scores) + attention_mask
    local = softmax(scores); attn = 0.5*local + 0.5*supplied
    attn = attn / (attn.sum(-1) + 1e-9); attn = where(causal, 0, attn)
    out = (attn @ v) @ Wo + bo

Sharding: 8 cores; core c handles batch b=c//2 and heads [8*(c%2), 8*(c%2)+8).
Each core returns a partial output [Q, DIM]; host sums the two head-half
partials per batch and adds bo.

Device algorithm (per core), f16 matmuls (1 col/cycle @2.4GHz) except the
supplied branch which runs fp8e4 DoubleRow (2 k-blocks per pass):
    - Projections consume host-transposed activations (x^T: [DIM, tok]) so
      Q^T [c,q], K^T [c,q] and V [k,c] come out of the PE in natural layout.
    - Attention runs in transposed layout S^T [k, q]; exp(S^T) is the moving
      operand for attn@V with V stationary; a ones-column appended to V
      accumulates E_q = sum_k exp for free.
    - Causal diag masking is an additive PE matmul (ident @ (-65504*tril))
      accumulated into the score PSUM before exp -> no DVE dependency.
    - supplied^T arrives as fp8e4, pre-scaled by 256*c2 and causally zeroed;
      sup@V uses DoubleRow (paired k-blocks) with an fp8 copy of V; the
      x256 is undone by passing Wo/256 (AT tiles carry 256x values).
    - Normalization: c1 = 256*c2/E via DVE row ops + gpsimd broadcast.
    - Phase interleaving: V-proj kb4-7 rides inside attention qc0; output
      projection for q<512 rides inside attention qc1; scores are issued two
      k-blocks ahead of their consumers to hide the scalar-engine exp.
"""

import numpy as np
from contextlib import ExitStack

B, Q, KLEN, DIM, H, D = 4, 1024, 1024, 1024, 16, 64
SCALE = float(D) ** -0.5
MIX = 0.5
NEG = -1.0e9
N_CORES = 8
NH = 8            # heads per core
P = 128
NKB = KLEN // P   # 8 k-blocks
QCH = 512         # q chunk (one PSUM bank of fp32)
SUPS = 256.0      # supplied-branch pre-scale (undone via Wo/SUPS)

_BUILD_CACHE = {}


def _build(causal: bool):
    """Build + compile the Bass program. causal=True: standard causal mask;
    causal=False: no masking at all."""
    import concourse.tile as tile
    import concourse.mybir as mybir
    from concourse import bacc

    F32 = mybir.dt.float32
    F16 = mybir.dt.float16
    F8 = mybir.dt.float8e4
    AF = mybir.ActivationFunctionType
    OP = mybir.AluOpType
    DR = mybir.MatmulPerfMode.DoubleRow

    nc = bacc.Bacc("TRN2", target_bir_lowering=False, debug=False,
                   num_devices=N_CORES)

    qT = nc.dram_tensor("qT", [DIM, Q], F16, kind="ExternalInput").ap()
    kT = nc.dram_tensor("kT", [DIM, KLEN], F16, kind="ExternalInput").ap()
    vT = nc.dram_tensor("vT", [DIM, KLEN], F16, kind="ExternalInput").ap()
    wq = nc.dram_tensor("wq", [DIM, NH * D], F16, kind="ExternalInput").ap()
    wk = nc.dram_tensor("wk", [DIM, NH * D], F16, kind="ExternalInput").ap()
    wv = nc.dram_tensor("wv", [DIM, NH * D], F16, kind="ExternalInput").ap()
    wo = nc.dram_tensor("wo", [NH * D, DIM], F16, kind="ExternalInput").ap()
    sup = nc.dram_tensor("sup", [NH, KLEN, Q], F8, kind="ExternalInput").ap()
    c2 = nc.dram_tensor("c2", [NH, Q], F32, kind="ExternalInput").ap()
    tri = nc.dram_tensor("tri", [P, P], F16, kind="ExternalInput").ap()
    idn = nc.dram_tensor("idn", [P, P], F16, kind="ExternalInput").ap()
    out = nc.dram_tensor("out_p", [Q, DIM], F16, kind="ExternalOutput").ap()

    def wlo_of(kb, qc):
        # start column of k-block kb's unmasked window, relative to chunk qc.
        if not causal:
            return 0
        return max(P * kb - qc * QCH, 0)

    with tile.TileContext(nc) as tc:
        with ExitStack() as ctx:
            # --- pools ---
            xT_pool = ctx.enter_context(tc.tile_pool(name="xT", bufs=3))
            w_pool = ctx.enter_context(tc.tile_pool(name="w", bufs=2))
            st_pool = ctx.enter_context(tc.tile_pool(name="store", bufs=1))
            sup_pool = ctx.enter_context(tc.tile_pool(name="sup", bufs=6))
            exp_pool = ctx.enter_context(tc.tile_pool(name="exp", bufs=5))
            row_pool = ctx.enter_context(tc.tile_pool(name="rows", bufs=2))
            rep_pool = ctx.enter_context(tc.tile_pool(name="rep", bufs=2))
            tmp_pool = ctx.enter_context(tc.tile_pool(name="tmp", bufs=2))
            const_pool = ctx.enter_context(tc.tile_pool(name="const", bufs=1))
            outb_pool = ctx.enter_context(tc.tile_pool(name="outb", bufs=2))

            # --- constants ---
            warm = const_pool.tile([P, QCH], F16, tag="warm")
            nc.vector.memset(warm[:], 0.0)
            if causal:
                tri_sb = const_pool.tile([P, P], F16, tag="tri")
                nc.sync.dma_start(tri_sb[:], tri)
                idn_sb = const_pool.tile([P, P], F16, tag="idn")
                nc.sync.dma_start(idn_sb[:], idn)

            # --- persistent stores ---
            # QT_st/KT_st tile j holds projected heads 2j,2j+1: [c=128, q=1024]
            QT_st = [st_pool.tile([P, Q], F16, tag=f"qt{j}", name=f"qt{j}")
                     for j in range(4)]
            KT_st = [st_pool.tile([P, Q], F16, tag=f"kt{j}", name=f"kt{j}")
                     for j in range(4)]
            # V_st[kb]: [k=128, NH*(D+1)] f16 (per head: D cols of V, ones col)
            V_st = [st_pool.tile([P, NH * (D + 1)], F16, tag=f"vst{kb}",
                                 name=f"vst{kb}") for kb in range(NKB)]
            # V8_st[pb]: [k=128, 2, NH*D] fp8 pair tiles for DoubleRow
            V8_st = [st_pool.tile([P, 2, NH * D], F8, tag=f"v8st{pb}",
                                  name=f"v8st{pb}") for pb in range(NKB // 2)]
            # attnT tile j: [hd=128 (heads 2j,2j+1), q=1024], carries 256x
            AT_st = [st_pool.tile([P, Q], F16, tag=f"at{j}", name=f"at{j}")
                     for j in range(4)]

            # ========== Phase 1a: Q/K projections ==========
            # ib-outer: 8 concurrent PSUM chains (all banks) so each DMA
            # chunk arrival immediately feeds 8 matmuls -> PE tracks the
            # cold-start DMA stream with minimal idle.
            with ExitStack() as pctx:
                proj_psum = pctx.enter_context(
                    tc.tile_pool(name="projpsum", bufs=8, space="PSUM"))
                # PE warmup (ramp the p-state while the first chunks land)
                wps = proj_psum.tile([P, QCH], F32, tag="c", name="wps")
                for _ in range(4):
                    nc.tensor.matmul(wps[:], warm[:, 0:P], warm[:],
                                     start=True, stop=True)
                for name, w_ap, x_ap, dst in (
                    ("q", wq, qT, QT_st), ("k", wk, kT, KT_st)):
                    w_sb = w_pool.tile([P, NKB, NH * D], F16, tag="w")
                    w_r = w_ap.rearrange("(n p) c -> p n c", p=P)
                    x_r = x_ap.rearrange("(n p) q -> p n q", p=P)
                    xh = [xT_pool.tile([P, 4, Q], F16, tag="xT",
                                       name=f"x{name}{half}")
                          for half in range(2)]
                    for ib in range(NKB):
                        nc.sync.dma_start(w_sb[:, ib:ib + 1, :],
                                          w_r[:, ib:ib + 1, :])
                        nc.sync.dma_start(
                            xh[ib // 4][:, ib % 4:ib % 4 + 1, :],
                            x_r[:, ib:ib + 1, :])
                    chains = [proj_psum.tile([P, QCH], F32, tag="c",
                                             name=f"{name}c{i}")
                              for i in range(8)]
                    for ib in range(NKB):
                        for ct in range(4):
                            for qc in range(2):
                                nc.tensor.matmul(
                                    chains[ct * 2 + qc][:],
                                    w_sb[:, ib, ct * P:(ct + 1) * P],
                                    xh[ib // 4][:, ib % 4,
                                                qc * QCH:(qc + 1) * QCH],
                                    start=(ib == 0), stop=(ib == NKB - 1))
                    for ct in range(4):
                        for qc in range(2):
                            nc.vector.tensor_copy(
                                out=dst[ct][:, qc * QCH:(qc + 1) * QCH],
                                in_=chains[ct * 2 + qc][:])

            s_psum = ctx.enter_context(
                tc.tile_pool(name="spsum", bufs=3, space="PSUM"))
            a_psum = ctx.enter_context(
                tc.tile_pool(name="apsum", bufs=2, space="PSUM"))
            b_psum = ctx.enter_context(
                tc.tile_pool(name="bpsum", bufs=2, space="PSUM"))

            # ========== Phase 1b: V projection (split around attention) ==
            wv_sb = w_pool.tile([P, NKB, NH * D], F16, tag="w")
            wv_r = wv.rearrange("(n p) c -> p n c", p=P)
            vT_r = vT.rearrange("(n p) q -> p n q", p=P)
            vh = [xT_pool.tile([P, 4, KLEN], F16, tag="xT", name=f"xv{half}")
                  for half in range(2)]
            for ib in range(NKB):
                nc.sync.dma_start(wv_sb[:, ib:ib + 1, :], wv_r[:, ib:ib + 1, :])
                nc.sync.dma_start(
                    vh[ib // 4][:, ib % 4:ib % 4 + 1, :],
                    vT_r[:, ib:ib + 1, :])

            def v_copies(kb, ps):
                # scatter per-head 64-col groups into the 65-stride layout
                nc.scalar.copy(
                    V_st[kb][:].rearrange("p (h x) -> p h x", x=D + 1)[:, :, 0:D],
                    ps[:].rearrange("p (h x) -> p h x", x=D))
                nc.vector.memset(
                    V_st[kb][:].rearrange("p (h x) -> p h x", x=D + 1)[:, :, D:D + 1],
                    1.0)
                nc.scalar.copy(V8_st[kb // 2][:, kb % 2, :], ps[:])

            def v_proj(kb):
                # V: out[k, c] += v^T[i, k]-as-lhsT @ Wv[i, c]
                ps = s_psum.tile([P, NH * D], F32, tag="s")
                for ib in range(NKB):
                    nc.tensor.matmul(
                        ps[:],
                        vh[ib // 4][:, ib % 4, kb * P:(kb + 1) * P],
                        wv_sb[:, ib, :],
                        start=(ib == 0), stop=(ib == NKB - 1))
                v_copies(kb, ps)

            # wo load early so phase 3 never waits
            wo_sb = w_pool.tile([P, 4, DIM], F16, tag="wow", name="wo_sb")
            nc.sync.dma_start(
                wo_sb[:], wo.rearrange("(n p) o -> p n o", p=P))

            n_vpre = 4 if causal else NKB
            for kb in range(n_vpre):
                v_proj(kb)

            # ========== Phase 2: attention ==========
            sup_r = sup.rearrange("h (n p) q -> h p n q", p=P)

            def attn_head(h, qc):
                j, po = h // 2, (h % 2) * D
                kmax = (4 * qc + 4) if causal else NKB
                npairs = kmax // 2
                cols = slice(qc * QCH, (qc + 1) * QCH)
                # supplied^T fp8 load: k-block pairs with causal windows
                sup_t = sup_pool.tile([P, kmax, QCH], F8, tag="sup")
                if causal and qc == 0:
                    for kb2 in range(0, 4, 2):
                        w2 = wlo_of(kb2, 0)
                        nc.sync.dma_start(
                            sup_t[:, kb2:kb2 + 2, w2:],
                            sup_r[h, :, kb2:kb2 + 2, w2:QCH])
                else:
                    nc.sync.dma_start(
                        sup_t[:, 0:kmax, :], sup_r[h, :, 0:kmax, cols])
                c2row = row_pool.tile([1, QCH], F32, tag="c2row")
                nc.sync.dma_start(c2row[:], c2[h:h + 1, cols])

                o2a = a_psum.tile([D + 1, QCH], F32, tag="o2a")
                o2b = b_psum.tile([D, QCH], F32, tag="o2b")

                s_ps = [None] * kmax
                e_t = [None] * kmax

                def score(kb):
                    wlo = wlo_of(kb, qc)
                    ps = s_psum.tile([P, QCH], F32, tag="s", name=f"sps{kb}")
                    s_ps[kb] = ps
                    diag = causal and (4 * qc <= kb < 4 * qc + 4)
                    nc.tensor.matmul(
                        ps[:, wlo:],
                        KT_st[j][po:po + D, kb * P:(kb + 1) * P],
                        QT_st[j][po:po + D, qc * QCH + wlo:(qc + 1) * QCH],
                        start=True, stop=not diag)
                    if diag:
                        dstart = kb * P - qc * QCH
                        nc.tensor.matmul(
                            ps[:, dstart:dstart + P], idn_sb[:], tri_sb[:],
                            start=False, stop=True, skip_group_check=True)
                    et = exp_pool.tile([P, QCH], F16, tag="e", name=f"et{kb}")
                    e_t[kb] = et
                    nc.scalar.activation(
                        et[:, wlo:], ps[:, wlo:], AF.Exp, bias=0.0, scale=SCALE)

                def o2a_mm(kb):
                    wlo = wlo_of(kb, qc)
                    nc.tensor.matmul(
                        o2a[:, wlo:],
                        V_st[kb][:, h * (D + 1):(h + 1) * (D + 1)],
                        e_t[kb][:, wlo:],
                        start=(kb == 0), stop=(kb == kmax - 1))
                    s_ps[kb] = None
                    e_t[kb] = None

                def o2b_mm(pb):
                    pw = wlo_of(2 * pb, qc)
                    nc.tensor.matmul(
                        o2b[:, pw:],
                        V8_st[pb][:, :, h * D:(h + 1) * D],
                        sup_t[:, 2 * pb:2 * pb + 2, pw:],
                        start=(pb == 0), stop=(pb == npairs - 1),
                        perf_mode=DR)

                # software pipeline: scores two blocks ahead; sup pairs fill
                # the exp latency between score(kb) and o2a(kb).
                score(0)
                score(1)
                for kb in range(kmax):
                    if kb % 2 == 0 and kb // 2 < npairs:
                        o2b_mm(kb // 2)
                    o2a_mm(kb)
                    if kb + 2 < kmax:
                        score(kb + 2)

                # c1 = 256*c2 / E ; attn^T = c1 (x) o2a[0:D] + o2b
                # (reciprocal_approx_fast reads garbage from PSUM; stage via
                # SBUF first)
                ecopy = row_pool.tile([1, QCH], F32, tag="ecopy")
                nc.vector.tensor_copy(out=ecopy[:], in_=o2a[D:D + 1, :])
                erec = row_pool.tile([1, QCH], F32, tag="erec")
                nc.vector.reciprocal_approx_fast(erec[:], ecopy[:])
                c1r = row_pool.tile([1, QCH], F32, tag="c1r")
                nc.vector.tensor_tensor(
                    out=c1r[:], in0=erec[:], in1=c2row[:], op=OP.mult)
                rep = rep_pool.tile([D, QCH], F32, tag="rep")
                nc.gpsimd.partition_broadcast(rep[:], c1r[:])
                t1 = tmp_pool.tile([D, QCH], F32, tag="t1")
                nc.vector.tensor_tensor(
                    out=t1[:], in0=o2a[0:D, :], in1=rep[:], op=OP.mult)
                nc.vector.tensor_tensor(
                    out=AT_st[j][po:po + D, cols], in0=t1[:],
                    in1=o2b[:], op=OP.add)

            # ========== Phase 3: output projection (per q-half) ==========
            def phase3_chunk(m, oc):
                ps = s_psum.tile([P, QCH], F32, tag="s")
                for jj in range(4):
                    nc.tensor.matmul(
                        ps[:],
                        AT_st[jj][:, m * P:(m + 1) * P],
                        wo_sb[:, jj, oc * QCH:(oc + 1) * QCH],
                        start=(jj == 0), stop=(jj == 3))
                ob = outb_pool.tile([P, QCH], F16, tag="ob")
                # alternate copy engine so the final drain is not serialized
                # on the scalar engine
                if (m + oc) % 2 == 0:
                    nc.scalar.copy(ob[:], ps[:])
                else:
                    nc.vector.tensor_copy(out=ob[:], in_=ps[:])
                nc.sync.dma_start(
                    out[m * P:(m + 1) * P, oc * QCH:(oc + 1) * QCH], ob[:])

            def phase3(qc):
                for m in range(4 * qc, 4 * qc + 4):
                    for oc in range(2):
                        phase3_chunk(m, oc)

            # attention qc0 with the tail of the V projection interleaved
            for h in range(NH):
                attn_head(h, 0)
                if causal and h < NKB - n_vpre:
                    v_proj(n_vpre + h)
            # attention qc1; output projection for q<512 spread across head
            # boundaries to smooth engine power (dense bursts trip the
            # throttle and drop the PE clock)
            # 6 chunks spread over h1..h6; 2 reserved to cover the last
            # head's c1-chain latency before phase3(1) can start
            p3q0 = [(m, oc) for m in range(4) for oc in range(2)]
            for h in range(NH):
                attn_head(h, 1)
                if 1 <= h <= 6:
                    phase3_chunk(*p3q0.pop(0))
            while p3q0:
                phase3_chunk(*p3q0.pop(0))
            phase3(1)

    nc.compile()
    return nc


def _prep_inputs(query, key, value, supplied_attn, Wq, Wk, Wv, Wo, causal):
    """Host-side marshaling: per-core transposed slices + normalization rows."""
    import ml_dtypes
    f32 = np.float32
    f16 = np.float16
    f8 = ml_dtypes.float8_e4m3
    # c2 = MIX / (MIX*sum(local) + (1-MIX)*sum(supplied) + 1e-9); sum(local)=1
    s_row = supplied_attn.sum(axis=-1, dtype=np.float32)          # [B,H,Q]
    denom = (MIX + (1.0 - MIX) * s_row + 1e-9).astype(f32)
    c2f = (np.float32(1.0 - MIX) / denom).astype(f32)             # [B,H,Q]
    c2_exp = (np.float32(MIX * SUPS) / denom).astype(f32)         # exp branch

    trif = np.tril(np.full((P, P), -65504.0, dtype=f16), -1)      # k>q rows
    idnf = np.eye(P, dtype=f16)

    in_maps = []
    for core in range(N_CORES):
        b, hh = core // 2, core % 2
        h0 = hh * NH
        qTa = np.ascontiguousarray(query[b].T.astype(f16))
        kTa = np.ascontiguousarray(key[b].T.astype(f16))
        vTa = np.ascontiguousarray(value[b].T.astype(f16))
        wqa = np.ascontiguousarray(Wq[:, h0 * D:(h0 + NH) * D].astype(f16))
        wka = np.ascontiguousarray(Wk[:, h0 * D:(h0 + NH) * D].astype(f16))
        wva = np.ascontiguousarray(Wv[:, h0 * D:(h0 + NH) * D].astype(f16))
        woa = np.ascontiguousarray(
            (Wo[h0 * D:(h0 + NH) * D, :] * (1.0 / SUPS)).astype(f16))
        s = supplied_attn[b, h0:h0 + NH]                          # [NH, Q, K]
        s = s * (c2f[b, h0:h0 + NH, :, None] * np.float32(SUPS))  # pre-scale
        if causal:
            s = np.tril(s)                                        # zero k>q
        supa = np.ascontiguousarray(s.transpose(0, 2, 1).astype(f8))  # [NH,K,Q]
        in_maps.append({
            "qT": qTa, "kT": kTa, "vT": vTa,
            "wq": wqa, "wk": wka, "wv": wva, "wo": woa,
            "sup": supa,
            "c2": np.ascontiguousarray(c2_exp[b, h0:h0 + NH], dtype=f32),
            "tri": trif, "idn": idnf,
        })
    return in_maps


def _fallback_numpy(query, key, value, attention_mask, supplied_attn,
                    Wq, Wk, Wv, Wo, bo, causal_mask):
    q = (query @ Wq).reshape(B, Q, H, D).transpose(0, 2, 1, 3)
    k = (key @ Wk).reshape(B, KLEN, H, D).transpose(0, 2, 1, 3)
    v = (value @ Wv).reshape(B, KLEN, H, D).transpose(0, 2, 1, 3)
    scores = np.einsum("bhqd,bhkd->bhqk", q, k).astype(np.float32) * np.float32(SCALE)
    cm = np.broadcast_to(causal_mask, scores.shape)
    scores = np.where(cm, np.float32(NEG), scores)
    scores = scores + attention_mask
    m = scores.max(axis=-1, keepdims=True)
    e = np.exp(scores - m)
    local = e / e.sum(axis=-1, keepdims=True)
    attn = np.float32(MIX) * local + np.float32(1.0 - MIX) * supplied_attn
    attn = attn / (attn.sum(axis=-1, keepdims=True) + np.float32(1e-9))
    attn = np.where(cm, np.float32(0.0), attn)
    o = np.einsum("bhqk,bhkd->bhqd", attn, v)
    o = o.transpose(0, 2, 1, 3).reshape(B, Q, H * D)
    return (o @ Wo + bo).astype(np.float32)


def kernel(query, key, value, attention_mask, supplied_attn,
           Wq, Wk, Wv, Wo, bo, causal_mask, _collect=None):
    query = np.asarray(query); key = np.asarray(key); value = np.asarray(value)
    attention_mask = np.asarray(attention_mask)
    supplied_attn = np.asarray(supplied_attn)
    Wq = np.asarray(Wq); Wk = np.asarray(Wk); Wv = np.asarray(Wv)
    Wo = np.asarray(Wo); bo = np.asarray(bo)
    causal_mask = np.asarray(causal_mask)

    cm2 = causal_mask.reshape(causal_mask.shape[-2], causal_mask.shape[-1])
    is_std_causal = bool(
        np.array_equal(cm2, np.triu(np.ones((Q, KLEN), dtype=bool), 1)))
    is_no_mask = not causal_mask.any()
    if attention_mask.any() or not (is_std_causal or is_no_mask):
        return _fallback_numpy(query, key, value, attention_mask,
                               supplied_attn, Wq, Wk, Wv, Wo, bo, causal_mask)

    import concourse.bass_utils as bass_utils
    causal = is_std_causal
    key_ = ("causal" if causal else "nomask")
    if key_ not in _BUILD_CACHE:
        _BUILD_CACHE[key_] = _build(causal)
    nc = _BUILD_CACHE[key_]

    in_maps = _prep_inputs(query, key, value, supplied_attn, Wq, Wk, Wv, Wo,
                           causal)
    run_kwargs = dict(_collect) if _collect else {}
    res = bass_utils.run_bass_kernel_spmd(
        nc, in_maps, core_ids=list(range(N_CORES)), **run_kwargs)
    if _collect is not None:
        _collect["results"] = res

    out = np.empty((B, Q, DIM), dtype=np.float32)
    for b in range(B):
        out[b] = (res.results[2 * b]["out_p"].astype(np.float32)
                  + res.results[2 * b + 1]["out_p"].astype(np.float32)
                  + bo.astype(np.float32))
    return out
